# revision 34
# baseline (speedup 1.0000x reference)
"""Trainium2 Bass kernel for nn_CrossAttentionMasked.

Reference computation (B=4, N=4096, M=1024, QD=640, KD=VD=768, H=8, C=80):
    q = x @ Wq; k = key @ Wk; v = value @ Wv       (per-head C=80)
    S = q k^T / sqrt(C); qmask = box_mask.reshape(B,N) > 0.5
    S masked rows -> uniform softmax, but post-attention masked_fill zeroes
    those rows anyway, so masked rows' output is exactly `bout`.
    out = softmax(S) @ v  (rows zeroed where ~qmask); y = out @ Wout + bout

Sharding: 8 cores = 4 batches x 2 head-halves (4 heads per core).
Host compacts unmasked query rows (~50% of 4096) and transposes activations;
device computes projections, attention with S stored transposed ([m, n]
layout so no on-chip transposes are needed), softmax denominator via a
ones-column in augmented V (row 96: engine PSUM reads need a 32-aligned
partition base), and the output projection.  Host sums the two head-half
partial outputs per batch, adds the output bias, and scatters into the
full result.

Structure (tuned against HW-probe attribution, not the cost model — the
ACT queue and per-instruction bubbles dominate on hardware):
  - S chunks land pairwise in a 2-bank PSUM tile so each softmax-exp
    covers 1024 free elements: halves the ACT instruction count, which is
    the most saturated engine queue (~0.3us fixed bubble per activation).
  - Next-group S/exp matmuls are interleaved chunk-wise with the current
    group AV accumulation (s_exp_av), so the in-order PE queue always has
    ready work between S chunks instead of idling at softmax-exp pace.
  - Each group normalize (Pool engine) + out-projection + PSUM->SBUF copy
    (ACT) + store is DEFERRED into the next group body: by then the
    softmax-reciprocal DRAM round-trip has landed, so nothing ever blocks
    an engine queue waiting on it.  The last (small) group instead runs a
    per-head tail pipeline with PE rank-1 reciprocal broadcasts.
  - The four per-head reciprocal rows stack in one [1, HPC, 512] tile so
    the round-trip is one store + one broadcast-load per group.
  - The timing loop body (reps>1) is unrolled 4x so For_i back-edge
    serialization is amortized; kernel() dispatches twice and accepts only
    bitwise-agreeing runs (transient-corruption guard).

All matmul operands are bf16 (fp32 PSUM accumulation).  Every DRAM tensor
is pre-laid-out partition-major on the host so each DMA is one contiguous
run per partition.
"""

from contextlib import ExitStack

import numpy as np
from ml_dtypes import bfloat16

import concourse.bass as bass
import concourse.mybir as mybir
import concourse.tile as tile
from concourse import bacc
from concourse.bass_utils import run_bass_kernel_spmd

B, N, M = 4, 4096, 1024
QD, KD, VD = 640, 768, 768
H, C = 8, 80
SIZE = 64
HPC = 4            # heads per core
CP = 128           # per-head channel dim padded 80 -> 128 (q/k projections)
VAUG = 97          # augmented v rows: 80 v chans + pad + denominator row at
                   # 96 (engine PSUM reads need a 32-aligned partition base)
SCALE = C ** -0.5
F32 = mybir.dt.float32
BF16 = mybir.dt.bfloat16
EXP = mybir.ActivationFunctionType.Exp
COPY = mybir.ActivationFunctionType.Copy
MUL = mybir.AluOpType.mult
ADD = mybir.AluOpType.add

DQ_CH = QD // 128  # 5
DK_CH = KD // 128  # 6
M_CH = M // 128    # 8
NPK = 3            # packed out tiles: 320 chans -> 128+128+64
# packed-channel pieces: (tile, part0, head, chan0, length)
PK_PIECES = []
for _h in range(HPC):
    _c0 = _h * 80
    while _c0 < (_h + 1) * 80:
        _t, _p = _c0 // 128, _c0 % 128
        _len = min(128 - _p, (_h + 1) * 80 - _c0)
        PK_PIECES.append((_t, _p, _h, _c0 - _h * 80, _len))
        _c0 += _len


def _pbcast(row_ap, nparts):
    """Partition-broadcast AP: replicate a DRAM row-block across nparts."""
    return bass.AP(tensor=row_ap.tensor, offset=row_ap.offset,
                   ap=[[0, nparts]] + [list(d) for d in row_ap.ap])


def build(npad, reps=1):
    """Build the per-core Bass program for NPAD compacted+padded queries.

    reps > 1 wraps the whole body in a hardware loop that re-runs the full
    computation (idempotent: same DRAM in/out each iteration) — used by the
    timing harness to amortize the fixed per-dispatch RPC overhead out of
    the hardware-time measurement.
    """
    nc = bacc.Bacc("TRN2", target_bir_lowering=False)
    ngr = -(-npad // 512)  # query groups of <=512

    xt = nc.dram_tensor("xt", [128, ngr, DQ_CH, 512], BF16,
                        kind="ExternalInput")
    kt = nc.dram_tensor("kt", [128, 2, DK_CH, 512], BF16,
                        kind="ExternalInput")
    vt = nc.dram_tensor("vt", [128, M_CH, DK_CH, 128], BF16,
                        kind="ExternalInput")
    wq = nc.dram_tensor("wq", [128, DQ_CH, HPC * CP], BF16,
                        kind="ExternalInput")
    wk = nc.dram_tensor("wk", [128, DK_CH, HPC * CP], BF16,
                        kind="ExternalInput")
    wv = nc.dram_tensor("wv", [128, DK_CH, HPC * VAUG], BF16,
                        kind="ExternalInput")
    wout = nc.dram_tensor("wout", [80, HPC, QD], BF16, kind="ExternalInput")
    y = nc.dram_tensor("y", [128, ngr, 4, QD], BF16, kind="ExternalOutput")

    groups = [(off, min(512, npad - off)) for off in range(0, npad, 512)]

    with TileKernel(nc) as tk:
        args = (xt, kt, vt, wq, wk, wv, wout, y, groups, npad)
        if reps > 1:
            # Unroll the timing loop body 4x: consecutive body copies
            # overlap (next copy's loads run under this copy's tail), so
            # any serialization at the For_i back-edge is paid once per 4
            # full computations instead of once per computation.
            unroll = 4 if reps >= 8 else 1
            with tk.tc.For_i(0, reps // unroll, 1,
                             hint_engines=(mybir.EngineType.PE,)):
                for _ in range(unroll):
                    tk.emit(*args)
            for _ in range(reps % unroll):
                tk.emit(*args)
        else:
            tk.emit(*args)
    nc.compile()
    return nc


class TileKernel:
    def __init__(self, nc):
        self.nc = nc
        self.ctx = ExitStack()

    def __enter__(self):
        self.tc = self.ctx.enter_context(tile.TileContext(self.nc))
        ctx, tc = self.ctx, self.tc
        self.res = ctx.enter_context(tc.tile_pool(name="resident", bufs=1))
        self.pin = ctx.enter_context(tc.tile_pool(name="pin", bufs=2))
        self.ap = ctx.enter_context(tc.tile_pool(name="ap", bufs=2))
        self.expp = ctx.enter_context(tc.tile_pool(name="expp", bufs=6))
        self.pps = ctx.enter_context(
            tc.tile_pool(name="pps", bufs=2, space="PSUM"))
        self.stp = ctx.enter_context(
            tc.tile_pool(name="stp", bufs=2, space="PSUM"))
        self.ovp = ctx.enter_context(
            tc.tile_pool(name="ovp", bufs=2, space="PSUM"))
        self.dsc = ctx.enter_context(
            tc.tile_pool(name="dsc", bufs=3, space="DRAM"))
        return self

    def __exit__(self, *exc):
        return self.ctx.__exit__(*exc)

    def emit(self, xt, kt, vt, wq, wk, wv, wout, y, groups, npad):
        nc, tc = self.nc, self.tc
        res, pin, ap, expp = self.res, self.pin, self.ap, self.expp
        pps, stp, ovp, dsc = self.pps, self.stp, self.ovp, self.dsc
        yp = pps  # projection + out-proj accumulators time-share 3 banks

        # persistent tensors
        q_heads = [res.tile([128, npad], BF16, tag=f"qT{h}", name=f"qT{h}")
                   for h in range(HPC)]
        k_heads = [res.tile([128, M], BF16, tag=f"kT{h}", name=f"kT{h}")
                   for h in range(HPC)]
        v_sb = res.tile([128, M_CH, HPC * VAUG], BF16, tag="v_sb", name="v_sb")
        wk_sb = res.tile([128, DK_CH, HPC * CP], BF16, tag="wk_sb", name="wk_sb")
        wv_sb = res.tile([128, DK_CH, HPC * VAUG], BF16, tag="wv_sb", name="wv_sb")
        wq_sb = res.tile([128, DQ_CH, HPC * CP], BF16, tag="wq_sb", name="wq_sb")
        wout_sb = res.tile([80, HPC, QD], BF16, tag="wout_sb", name="wout_sb")

        # weight/bias loads — all on the sync queue, in need order, so the
        # DMA engine transfers wk+ksl0 first and the first matmul starts
        # ASAP (a second queue would race its transfers ahead of ksl0).
        nc.sync.dma_start(out=wk_sb[:], in_=wk[:])
        ksls = []
        for mg in range(2):
            ksl = pin.tile([128, DK_CH, 512], BF16, tag="ksl", name="ksl")
            nc.sync.dma_start(out=ksl[:], in_=kt[:, mg])
            ksls.append(ksl)
            if mg == 0:
                nc.sync.dma_start(out=wq_sb[:], in_=wq[:])

        def kproj(h):
            for mg in range(2):
                ps = pps.tile([128, 512], F32, tag="pp", name="pp")
                for dc in range(DK_CH):
                    nc.tensor.matmul(
                        ps[:], wk_sb[:, dc, h * CP:(h + 1) * CP],
                        ksls[mg][:, dc, :],
                        start=(dc == 0), stop=(dc == DK_CH - 1))
                nc.vector.tensor_copy(
                    k_heads[h][:, mg * 512:(mg + 1) * 512], ps[:])

        def vproj():
            # v projection: natural [m, head-aug channels]
            nc.sync.dma_start(out=wv_sb[:], in_=wv[:])
            vsl = pin.tile([128, M_CH, DK_CH, 128], BF16, tag="vsl",
                           name="vsl")
            nc.sync.dma_start(out=vsl[:], in_=vt[:])
            nc.sync.dma_start(out=wout_sb[:], in_=wout[:])
            for mc in range(M_CH):
                ps = pps.tile([128, 512], F32, tag="pp", name="pp")
                for dc in range(DK_CH):
                    nc.tensor.matmul(
                        ps[:, :HPC * VAUG], vsl[:, mc, dc, :],
                        wv_sb[:, dc, :],
                        start=(dc == 0), stop=(dc == DK_CH - 1))
                nc.vector.tensor_copy(v_sb[:, mc, :],
                                      ps[:, :HPC * VAUG])
            # softmax-denominator ones column per head (after all v copies)
            for h in range(HPC):
                nc.vector.tensor_copy(
                    v_sb[:, :, h * VAUG + 96:h * VAUG + 97],
                    nc.const_aps.tensor(1.0, (128, M_CH, 1), F32))

        # ---- per query group: q-proj, attention, out-proj ----
        # q-proj runs one group ahead of attention (software pipeline) so
        # its matmuls fill PE gaps while ACT paces the previous group.
        # v-proj is emitted AFTER q-proj(0): group 0's S/softmax-exp only
        # needs K and Q, so ACT starts ~20us earlier, overlapping v-proj.
        def qload(gi, gsz):
            xq = pin.tile([128, DQ_CH, 512], BF16, tag="xq", name="xq")
            nc.sync.dma_start(out=xq[:, :, :gsz], in_=xt[:, gi, :, :gsz])
            return xq

        def qproj_head(xq, h, g0, gsz):
            ps = pps.tile([128, 512], F32, tag="pp", name="pp")
            for dc in range(DQ_CH):
                nc.tensor.matmul(
                    ps[:, :gsz], wq_sb[:, dc, h * CP:(h + 1) * CP],
                    xq[:, dc, :gsz],
                    start=(dc == 0), stop=(dc == DQ_CH - 1))
            nc.vector.tensor_copy(q_heads[h][:, g0:g0 + gsz], ps[:, :gsz])

        def qproj(gi, g0, gsz):
            xq = qload(gi, gsz)
            for h in range(HPC):
                qproj_head(xq, h, g0, gsz)

        def s_exp(h, g0, gsz):
            # S chunk pairs in a 2-bank PSUM tile; one softmax-exp per pair
            # halves the ACT instruction count (ACT is the most saturated
            # engine: ~0.3us fixed bubble per activation instruction)
            expst = expp.tile([128, M_CH, 512], BF16, tag="expst",
                              name="expst")
            for t in range(M_CH // 2):
                st = stp.tile([128, 2, 512], F32, tag="st", name="st")
                for j in range(2):
                    mc = 2 * t + j
                    nc.tensor.matmul(
                        st[:, j, :gsz],
                        k_heads[h][:, mc * 128:(mc + 1) * 128],
                        q_heads[h][:, g0:g0 + gsz], start=True, stop=True)
                nc.scalar.activation(expst[:, 2 * t:2 * t + 2, :gsz],
                                     st[:, :, :gsz], EXP, scale=SCALE)
            return expst

        # ---- prologue: per head, k-proj -> q-proj(g0) -> S/exp(g0), so
        # the first softmax-exp lands on ACT ~10us in; v-proj follows. ----
        g0_0, gsz_0 = groups[0]
        xq0 = qload(0, gsz_0)
        expsts0 = []
        for h in range(HPC):
            kproj(h)
            qproj_head(xq0, h, g0_0, gsz_0)
            expsts0.append(s_exp(h, g0_0, gsz_0))

        expsts = {0: expsts0}
        pending = [None]  # deferred normalize+out-proj closure for group g-1

        def make_finisher(gi, g0, gsz, o_uns, bcast):
            """Normalize (Pool), out-project, PSUM->SBUF copy (ACT) and
            store group gi.  Runs inside group gi+1's body, after its
            h-loop: by then the reciprocal round-trip for gi has long
            landed, so nothing here blocks the DVE/ACT queues, and the
            PE matmuls act as filler between AV streams."""
            def fin():
                out_hs = []
                for h in range(HPC):
                    out_h = ap.tile([80, 512], BF16, tag=f"outH{h}",
                                    name=f"outH{h}")
                    nc.gpsimd.tensor_tensor(
                        out_h[:, :gsz], o_uns[h][:80, :gsz],
                        bcast[:, h, :gsz], MUL)
                    out_hs.append(out_h)
                ntile = gsz // 128
                ysb = ap.tile([128, 4, QD], BF16, tag="ysb", name="ysb")
                for nt0 in range(ntile):
                    for di in range(2):
                        yps = yp.tile([128, 512], F32, tag="pp", name="y")
                        for h in range(HPC):
                            nc.tensor.matmul(
                                yps[:, :320],
                                out_hs[h][:, nt0 * 128:(nt0 + 1) * 128],
                                wout_sb[:, h, di * 320:(di + 1) * 320],
                                start=(h == 0), stop=(h == HPC - 1))
                        nc.scalar.activation(
                            ysb[:, nt0, di * 320:(di + 1) * 320],
                            yps[:, :320], COPY)
                nc.sync.dma_start(out=y[:, gi, :ntile, :],
                                  in_=ysb[:, :ntile, :])
            return fin

        for gi, (g0, gsz) in enumerate(groups):
            # attention: unnormalized out + denominator per head, then one
            # batched reciprocal round-trip + partition-broadcast per group
            # (per-head PE-broadcast chains for the last group, to shorten
            # the exposed tail after the final AV)
            last = gi == len(groups) - 1
            bcast = ap.tile([80, HPC, 512], BF16, tag="bcast", name="bcast")
            rdr = dsc.tile([HPC, 512], BF16, tag="rdr", name="rdr")
            recip_all = ap.tile([1, HPC, 512], BF16, tag="recip",
                                name="recip_all")

            def av_norm(h, expst):
                oaug = ovp.tile([VAUG, 512], F32, tag="oaug", name="oaug")
                for mc in range(M_CH):
                    nc.tensor.matmul(
                        oaug[:, :gsz],
                        v_sb[:, mc, h * VAUG:(h + 1) * VAUG],
                        expst[:, mc, :gsz],
                        start=(mc == 0), stop=(mc == M_CH - 1))
                o_un = ap.tile([80, 512], F32, tag=f"oun{h}", name=f"oun{h}")
                nc.vector.tensor_copy(o_un[:, :gsz], oaug[:80, :gsz])
                with nc.allow_low_precision(reason="bf16 softmax recip"):
                    nc.vector.reciprocal(recip_all[:, h, :gsz],
                                         oaug[96:97, :gsz])
                o_uns.append(o_un)

            def s_exp_av(h, g0n, gszn, expst_cur):
                # next-group S/exp interleaved chunk-wise with the current
                # group's AV accumulation: the AV matmuls (whose exp inputs
                # were produced a whole group ago) are guaranteed PE filler
                # between S chunks, so the in-order PE queue never idles at
                # the softmax-exp pace of the stp-buffer rotation.
                expst = expp.tile([128, M_CH, 512], BF16, tag="expst",
                                  name="expst")
                oaug = ovp.tile([VAUG, 512], F32, tag="oaug", name="oaug")
                for t in range(M_CH // 2):
                    st = stp.tile([128, 2, 512], F32, tag="st", name="st")
                    for j in range(2):
                        mc = 2 * t + j
                        nc.tensor.matmul(
                            st[:, j, :gszn],
                            k_heads[h][:, mc * 128:(mc + 1) * 128],
                            q_heads[h][:, g0n:g0n + gszn],
                            start=True, stop=True, skip_group_check=True)
                        nc.tensor.matmul(
                            oaug[:, :gsz],
                            v_sb[:, mc, h * VAUG:(h + 1) * VAUG],
                            expst_cur[:, mc, :gsz],
                            start=(mc == 0), stop=(mc == M_CH - 1),
                            skip_group_check=True)
                    nc.scalar.activation(expst[:, 2 * t:2 * t + 2, :gszn],
                                         st[:, :, :gszn], EXP, scale=SCALE)
                o_un = ap.tile([80, 512], F32, tag=f"oun{h}",
                               name=f"oun{h}")
                nc.vector.tensor_copy(o_un[:, :gsz], oaug[:80, :gsz])
                with nc.allow_low_precision(reason="bf16 softmax recip"):
                    nc.vector.reciprocal(recip_all[:, h, :gsz],
                                         oaug[96:97, :gsz])
                o_uns.append(o_un)
                return expst

            def emit_bc(h):
                # PE rank-1 broadcast: PSUM[80,512] = ones^T @ recip —
                # PE is idle in the tail and this skips the DRAM
                # round-trip on the exposed critical path
                bc = ovp.tile([128, 512], F32, tag="oaug", name="bc")
                nc.tensor.matmul(
                    bc[:80, :gsz],
                    nc.const_aps.tensor(1.0, (1, 80), BF16),
                    recip_all[:, h, :gsz], start=True, stop=True)
                bcs.append(bc)

            # the deferred finisher for the previous group must run BEFORE
            # the tailpipe grabs both out-proj PSUM slots (deadlock), and by
            # the last group its reciprocal broadcast has long arrived
            if last and pending[0] is not None:
                pending[0]()
                pending[0] = None

            # last-group tail pipeline (ntile==1): after AV(h), finish head
            # h-1 (bc -> mul -> its two yout accumulation matmuls), so the
            # recip->bc->mul chain of head h overlaps AV(h+1) instead of
            # stalling PE, and only head 3's chain is tail-exposed.
            tailpipe = last and gsz == 128
            yps_d = ([yp.tile([128, 512], F32, tag="pp", name="y")
                      for _ in range(2)] if tailpipe else None)
            out_hs = []

            def finish_head(h):
                out_h = ap.tile([80, 512], BF16, tag=f"outH{h}",
                                name=f"outH{h}")
                nc.vector.tensor_tensor(
                    out_h[:, :gsz], o_uns[h][:80, :gsz],
                    bcs[h][:80, :gsz], MUL)
                out_hs.append(out_h)
                for di in range(2):
                    nc.tensor.matmul(
                        yps_d[di][:, :320], out_h[:, :128],
                        wout_sb[:, h, di * 320:(di + 1) * 320],
                        start=(h == 0), stop=(h == HPC - 1),
                        skip_group_check=True)

            o_uns, bcs = [], []
            if gi == 0:
                # S/exp for group 0 was emitted in the prologue; v-proj
                # (which AV needs) comes next, overlapping those exps,
                # then q-proj for group 1.
                vproj()
                if len(groups) > 1:
                    qproj(1, *groups[1])
            # software pipeline: S/exp for group g+1 interleaves per-head
            # with AV of group g on the in-order PE queue; the deferred
            # finisher for group g-1 and q-proj for g+2 follow as filler.
            if gi + 1 < len(groups):
                expsts[gi + 1] = []
                g1, s1 = groups[gi + 1]
            for h in range(HPC):
                if gi + 1 < len(groups):
                    expsts[gi + 1].append(
                        s_exp_av(h, g1, s1, expsts[gi][h]))
                else:
                    av_norm(h, expsts[gi][h])
                if tailpipe and h > 0:
                    emit_bc(h - 1)
                    finish_head(h - 1)
            if not last:
                nc.sync.dma_start(out=rdr[:, :gsz],
                                  in_=recip_all[:, :, :gsz])
                nc.sync.dma_start(out=bcast[:, :, :gsz],
                                  in_=_pbcast(rdr[:, :gsz], 80))
            if pending[0] is not None:
                pending[0]()
                pending[0] = None
            if gi + 2 < len(groups):
                qproj(gi + 2, *groups[gi + 2])

            if tailpipe:
                ysb = ap.tile([128, 4, QD], BF16, tag="ysb", name="ysb")
                emit_bc(HPC - 1)
                finish_head(HPC - 1)
                for di in range(2):
                    nc.scalar.activation(
                        ysb[:, 0, di * 320:(di + 1) * 320],
                        yps_d[di][:, :320], COPY)
                nc.sync.dma_start(out=y[:, gi, 0:1, :], in_=ysb[:, 0:1, :])
            elif last:
                # non-128 last group: finish inline via PE broadcasts
                ysb = ap.tile([128, 4, QD], BF16, tag="ysb", name="ysb")
                for h in range(HPC):
                    emit_bc(h)
                    out_h = ap.tile([80, 512], BF16, tag=f"outH{h}",
                                    name=f"outH{h}")
                    nc.vector.tensor_tensor(
                        out_h[:, :gsz], o_uns[h][:80, :gsz],
                        bcs[h][:80, :gsz], MUL)
                    out_hs.append(out_h)
                ntile = gsz // 128
                for nt0 in range(ntile):
                    for di in range(2):
                        yps = yp.tile([128, 512], F32, tag="pp", name="y")
                        for h in range(HPC):
                            nc.tensor.matmul(
                                yps[:, :320],
                                out_hs[h][:, nt0 * 128:(nt0 + 1) * 128],
                                wout_sb[:, h, di * 320:(di + 1) * 320],
                                start=(h == 0), stop=(h == HPC - 1))
                        nc.scalar.activation(
                            ysb[:, nt0, di * 320:(di + 1) * 320],
                            yps[:, :320], COPY)
                    nc.sync.dma_start(out=y[:, gi, nt0:nt0 + 1, :],
                                      in_=ysb[:, nt0:nt0 + 1, :])
            else:
                pending[0] = make_finisher(gi, g0, gsz, o_uns, bcast)
        assert pending[0] is None


def _pm(a, nchunk, p=128):
    """[nchunk*p, F...] -> partition-major [p, nchunk, F...]."""
    return np.ascontiguousarray(
        a.reshape(nchunk, p, *a.shape[1:]).transpose(
            1, 0, *range(2, a.ndim + 1)))


def _prep_core_inputs(x, key, value, wq, wk, wv, wout, bout,
                      qmask_idx, npad):
    """Host-side shard prep: returns list of 8 in_maps (bf16, partition-
    major layouts matching the DRAM tensor declarations in build())."""
    ngr = -(-npad // 512)
    in_maps = []
    xt_b, kt_b, vt_b = {}, {}, {}
    for b in range(B):
        idx = qmask_idx[b]
        xs = np.zeros((QD, ngr * 512), dtype=bfloat16)
        xs[:, :len(idx)] = np.ascontiguousarray(x[b][idx].T).astype(bfloat16)
        # [640, ngr*512] -> [5, 128, ngr, 512] -> [128, ngr, 5, 512]
        xt_b[b] = np.ascontiguousarray(
            xs.reshape(DQ_CH, 128, ngr, 512).transpose(1, 2, 0, 3))
        kb = np.ascontiguousarray(key[b].T).astype(bfloat16)
        kt_b[b] = np.ascontiguousarray(
            kb.reshape(DK_CH, 128, 2, 512).transpose(1, 2, 0, 3))
        vb = np.ascontiguousarray(value[b].T).astype(bfloat16)
        vt_b[b] = np.ascontiguousarray(
            vb.reshape(DK_CH, 128, M_CH, 128).transpose(1, 2, 0, 3))

    w_half = {}
    for hh in range(2):
        wq_a = np.zeros((QD, HPC * CP), dtype=bfloat16)
        wk_a = np.zeros((KD, HPC * CP), dtype=bfloat16)
        wv_a = np.zeros((KD, HPC * VAUG), dtype=bfloat16)
        for hp in range(HPC):
            hg = hh * HPC + hp
            wq_a[:, hp * CP:hp * CP + 80] = wq[:, hg * 80:(hg + 1) * 80].astype(bfloat16)
            wk_a[:, hp * CP:hp * CP + 80] = wk[:, hg * 80:(hg + 1) * 80].astype(bfloat16)
            wv_a[:, hp * VAUG:hp * VAUG + 80] = wv[:, hg * 80:(hg + 1) * 80].astype(bfloat16)
        w_half[hh] = (_pm(wq_a, DQ_CH), _pm(wk_a, DK_CH), _pm(wv_a, DK_CH))

    for core in range(8):
        b, hh = core // 2, core % 2
        wq_a, wk_a, wv_a = w_half[hh]
        wo = wout[hh * HPC * 80:(hh + 1) * HPC * 80].astype(bfloat16)
        wout_a = np.ascontiguousarray(
            wo.reshape(HPC, 80, QD).transpose(1, 0, 2))
        in_maps.append({
            "xt": xt_b[b], "kt": kt_b[b], "vt": vt_b[b],
            "wq": wq_a, "wk": wk_a, "wv": wv_a, "wout": wout_a,
        })
    return in_maps


def kernel(x, key, value, box_mask, Wq, Wk, Wv, Wout, bout, _trace=False):
    x = np.asarray(x, dtype=np.float32)
    key = np.asarray(key, dtype=np.float32)
    value = np.asarray(value, dtype=np.float32)
    box_mask = np.asarray(box_mask, dtype=np.float32)
    Wq, Wk, Wv = (np.asarray(a, dtype=np.float32) for a in (Wq, Wk, Wv))
    Wout = np.asarray(Wout, dtype=np.float32)
    bout = np.asarray(bout, dtype=np.float32)

    qmask = box_mask[:, 0].reshape(B, N) > 0.5
    qmask_idx = [np.nonzero(qmask[b])[0] for b in range(B)]
    cnt_max = max(1, max(len(i) for i in qmask_idx))
    npad = -(-cnt_max // 128) * 128

    nc = build(npad)
    in_maps = _prep_core_inputs(x, key, value, Wq, Wk, Wv, Wout, bout,
                                qmask_idx, npad)
    # Device execution with transient-corruption guard: dispatch twice and
    # accept only when two runs agree bitwise (HW is deterministic on clean
    # runs; a glitched run differs).  Retry a few times on mismatch.
    kr = run_bass_kernel_spmd(nc, in_maps, core_ids=list(range(8)),
                              trace=_trace)
    results = kr.results
    if not _trace:
        prev = [r["y"].copy() for r in results]
        for _attempt in range(4):
            kr2 = run_bass_kernel_spmd(nc, in_maps, core_ids=list(range(8)))
            cur = [r["y"] for r in kr2.results]
            if all(np.array_equal(a, b) for a, b in zip(prev, cur)):
                results = kr2.results
                break
            prev = [c.copy() for c in cur]
        else:
            results = kr2.results

    out = np.broadcast_to(bout, (B, N, QD)).copy().astype(np.float32)
    for b in range(B):
        idx = qmask_idx[b]
        # y is [128, ngr, 4, QD]: query g*512 + nt0*128 + p lives at
        # y[p, g, nt0]
        y0 = results[2 * b]["y"].transpose(1, 2, 0, 3).reshape(-1, QD)
        y1 = results[2 * b + 1]["y"].transpose(1, 2, 0, 3).reshape(-1, QD)
        out[b][idx] = (y0[:len(idx)].astype(np.float32)
                       + y1[:len(idx)].astype(np.float32) + bout)
    if _trace:
        return out, kr
    return out



# revision 35
# speedup vs baseline: 1.0048x; 1.0048x over previous
"""Trainium2 Bass kernel for nn_CrossAttentionMasked.

Reference computation (B=4, N=4096, M=1024, QD=640, KD=VD=768, H=8, C=80):
    q = x @ Wq; k = key @ Wk; v = value @ Wv       (per-head C=80)
    S = q k^T / sqrt(C); qmask = box_mask.reshape(B,N) > 0.5
    S masked rows -> uniform softmax, but post-attention masked_fill zeroes
    those rows anyway, so masked rows' output is exactly `bout`.
    out = softmax(S) @ v  (rows zeroed where ~qmask); y = out @ Wout + bout

Sharding: 8 cores = 4 batches x 2 head-halves (4 heads per core).
Host compacts unmasked query rows (~50% of 4096) and transposes activations;
device computes projections, attention with S stored transposed ([m, n]
layout so no on-chip transposes are needed), softmax denominator via a
ones-column in augmented V (row 96: engine PSUM reads need a 32-aligned
partition base), and the output projection.  Host sums the two head-half
partial outputs per batch, adds the output bias, and scatters into the
full result.

Structure (tuned against HW-probe attribution, not the cost model — the
ACT queue and per-instruction bubbles dominate on hardware):
  - S chunks land pairwise in a 2-bank PSUM tile so each softmax-exp
    covers 1024 free elements: halves the ACT instruction count, which is
    the most saturated engine queue (~0.3us fixed bubble per activation).
  - Next-group S/exp matmuls are interleaved chunk-wise with the current
    group AV accumulation (s_exp_av), so the in-order PE queue always has
    ready work between S chunks instead of idling at softmax-exp pace.
  - Each group normalize (Pool engine) + out-projection + PSUM->SBUF copy
    (ACT) + store is DEFERRED into the next group body: by then the
    softmax-reciprocal DRAM round-trip has landed, so nothing ever blocks
    an engine queue waiting on it.  The last (small) group instead runs a
    per-head tail pipeline with PE rank-1 reciprocal broadcasts.
  - The four per-head reciprocal rows stack in one [1, HPC, 512] tile so
    the round-trip is one store + one broadcast-load per group.
  - The timing loop body (reps>1) is unrolled 4x so For_i back-edge
    serialization is amortized; kernel() dispatches twice and accepts only
    bitwise-agreeing runs (transient-corruption guard).

All matmul operands are bf16 (fp32 PSUM accumulation).  Every DRAM tensor
is pre-laid-out partition-major on the host so each DMA is one contiguous
run per partition.
"""

from contextlib import ExitStack

import numpy as np
from ml_dtypes import bfloat16

import concourse.bass as bass
import concourse.mybir as mybir
import concourse.tile as tile
from concourse import bacc
from concourse.bass_utils import run_bass_kernel_spmd

B, N, M = 4, 4096, 1024
QD, KD, VD = 640, 768, 768
H, C = 8, 80
SIZE = 64
HPC = 4            # heads per core
CP = 128           # per-head channel dim padded 80 -> 128 (q/k projections)
VAUG = 97          # augmented v rows: 80 v chans + pad + denominator row at
                   # 96 (engine PSUM reads need a 32-aligned partition base)
SCALE = C ** -0.5
F32 = mybir.dt.float32
BF16 = mybir.dt.bfloat16
EXP = mybir.ActivationFunctionType.Exp
COPY = mybir.ActivationFunctionType.Copy
MUL = mybir.AluOpType.mult
ADD = mybir.AluOpType.add

DQ_CH = QD // 128  # 5
DK_CH = KD // 128  # 6
M_CH = M // 128    # 8
NPK = 3            # packed out tiles: 320 chans -> 128+128+64
# packed-channel pieces: (tile, part0, head, chan0, length)
PK_PIECES = []
for _h in range(HPC):
    _c0 = _h * 80
    while _c0 < (_h + 1) * 80:
        _t, _p = _c0 // 128, _c0 % 128
        _len = min(128 - _p, (_h + 1) * 80 - _c0)
        PK_PIECES.append((_t, _p, _h, _c0 - _h * 80, _len))
        _c0 += _len


def _pbcast(row_ap, nparts):
    """Partition-broadcast AP: replicate a DRAM row-block across nparts."""
    return bass.AP(tensor=row_ap.tensor, offset=row_ap.offset,
                   ap=[[0, nparts]] + [list(d) for d in row_ap.ap])


def build(npad, reps=1):
    """Build the per-core Bass program for NPAD compacted+padded queries.

    reps > 1 wraps the whole body in a hardware loop that re-runs the full
    computation (idempotent: same DRAM in/out each iteration) — used by the
    timing harness to amortize the fixed per-dispatch RPC overhead out of
    the hardware-time measurement.
    """
    nc = bacc.Bacc("TRN2", target_bir_lowering=False)
    ngr = -(-npad // 512)  # query groups of <=512

    xt = nc.dram_tensor("xt", [128, ngr, DQ_CH, 512], BF16,
                        kind="ExternalInput")
    kt = nc.dram_tensor("kt", [128, 2, DK_CH, 512], BF16,
                        kind="ExternalInput")
    vt = nc.dram_tensor("vt", [128, M_CH, DK_CH, 128], BF16,
                        kind="ExternalInput")
    wq = nc.dram_tensor("wq", [128, DQ_CH, HPC * CP], BF16,
                        kind="ExternalInput")
    wk = nc.dram_tensor("wk", [128, DK_CH, HPC * CP], BF16,
                        kind="ExternalInput")
    wv = nc.dram_tensor("wv", [128, DK_CH, HPC * 80], BF16,
                        kind="ExternalInput")
    wout = nc.dram_tensor("wout", [80, HPC, QD], BF16, kind="ExternalInput")
    y = nc.dram_tensor("y", [128, ngr, 4, QD], BF16, kind="ExternalOutput")

    groups = [(off, min(512, npad - off)) for off in range(0, npad, 512)]

    with TileKernel(nc) as tk:
        args = (xt, kt, vt, wq, wk, wv, wout, y, groups, npad)
        if reps > 1:
            # Unroll the timing loop body 4x: consecutive body copies
            # overlap (next copy's loads run under this copy's tail), so
            # any serialization at the For_i back-edge is paid once per 4
            # full computations instead of once per computation.
            unroll = 4 if reps >= 8 else 1
            with tk.tc.For_i(0, reps // unroll, 1,
                             hint_engines=(mybir.EngineType.PE,)):
                for _ in range(unroll):
                    tk.emit(*args)
            for _ in range(reps % unroll):
                tk.emit(*args)
        else:
            tk.emit(*args)
    nc.compile()
    return nc


class TileKernel:
    def __init__(self, nc):
        self.nc = nc
        self.ctx = ExitStack()

    def __enter__(self):
        self.tc = self.ctx.enter_context(tile.TileContext(self.nc))
        ctx, tc = self.ctx, self.tc
        self.res = ctx.enter_context(tc.tile_pool(name="resident", bufs=1))
        self.pin = ctx.enter_context(tc.tile_pool(name="pin", bufs=2))
        self.ap = ctx.enter_context(tc.tile_pool(name="ap", bufs=2))
        self.expp = ctx.enter_context(tc.tile_pool(name="expp", bufs=6))
        self.pps = ctx.enter_context(
            tc.tile_pool(name="pps", bufs=2, space="PSUM"))
        self.stp = ctx.enter_context(
            tc.tile_pool(name="stp", bufs=2, space="PSUM"))
        self.ovp = ctx.enter_context(
            tc.tile_pool(name="ovp", bufs=2, space="PSUM"))
        self.dsc = ctx.enter_context(
            tc.tile_pool(name="dsc", bufs=3, space="DRAM"))
        return self

    def __exit__(self, *exc):
        return self.ctx.__exit__(*exc)

    def emit(self, xt, kt, vt, wq, wk, wv, wout, y, groups, npad):
        nc, tc = self.nc, self.tc
        res, pin, ap, expp = self.res, self.pin, self.ap, self.expp
        pps, stp, ovp, dsc = self.pps, self.stp, self.ovp, self.dsc
        yp = pps  # projection + out-proj accumulators time-share 3 banks

        # persistent tensors
        q_heads = [res.tile([128, npad], BF16, tag=f"qT{h}", name=f"qT{h}")
                   for h in range(HPC)]
        k_heads = [res.tile([128, M], BF16, tag=f"kT{h}", name=f"kT{h}")
                   for h in range(HPC)]
        v_sb = res.tile([128, M_CH, HPC * VAUG], BF16, tag="v_sb", name="v_sb")
        wk_sb = res.tile([128, DK_CH, HPC * CP], BF16, tag="wk_sb", name="wk_sb")
        wv_sb = res.tile([128, DK_CH, HPC * 80], BF16, tag="wv_sb", name="wv_sb")
        wq_sb = res.tile([128, DQ_CH, HPC * CP], BF16, tag="wq_sb", name="wq_sb")
        wout_sb = res.tile([80, HPC, QD], BF16, tag="wout_sb", name="wout_sb")

        # weight/bias loads — all on the sync queue, in need order, so the
        # DMA engine transfers wk+ksl0 first and the first matmul starts
        # ASAP (a second queue would race its transfers ahead of ksl0).
        nc.sync.dma_start(out=wk_sb[:], in_=wk[:])
        ksls = []
        for mg in range(2):
            ksl = pin.tile([128, DK_CH, 512], BF16, tag="ksl", name="ksl")
            nc.sync.dma_start(out=ksl[:], in_=kt[:, mg])
            ksls.append(ksl)
            if mg == 0:
                nc.sync.dma_start(out=wq_sb[:], in_=wq[:])

        def kproj(h):
            for mg in range(2):
                ps = pps.tile([128, 512], F32, tag="pp", name="pp")
                for dc in range(DK_CH):
                    nc.tensor.matmul(
                        ps[:], wk_sb[:, dc, h * CP:(h + 1) * CP],
                        ksls[mg][:, dc, :],
                        start=(dc == 0), stop=(dc == DK_CH - 1))
                nc.vector.tensor_copy(
                    k_heads[h][:, mg * 512:(mg + 1) * 512], ps[:])

        def vproj():
            # v projection: natural [m, head-aug channels]
            nc.sync.dma_start(out=wv_sb[:], in_=wv[:])
            vsl = pin.tile([128, M_CH, DK_CH, 128], BF16, tag="vsl",
                           name="vsl")
            nc.sync.dma_start(out=vsl[:], in_=vt[:])
            nc.sync.dma_start(out=wout_sb[:], in_=wout[:])
            for mc in range(M_CH):
                # tight 320-col projection (no 97-wide pad in the weights);
                # the copy scatters [4,80] head blocks into the 97-stride
                # v_sb layout via a strided free AP
                ps = pps.tile([128, 512], F32, tag="pp", name="pp")
                for dc in range(DK_CH):
                    nc.tensor.matmul(
                        ps[:, :HPC * 80], vsl[:, mc, dc, :],
                        wv_sb[:, dc, :],
                        start=(dc == 0), stop=(dc == DK_CH - 1))
                ps320 = ps[:, :HPC * 80]
                ps_h = bass.AP(tensor=ps320.tensor, offset=ps320.offset,
                               ap=[list(ps320.ap[0]), [80, HPC], [1, 80]])
                vslc = v_sb[:, mc, :]
                v_h = bass.AP(tensor=vslc.tensor, offset=vslc.offset,
                              ap=[list(vslc.ap[0]), [VAUG, HPC], [1, 80]])
                nc.vector.tensor_copy(v_h, ps_h)
            # softmax-denominator ones column per head (after all v copies)
            for h in range(HPC):
                nc.vector.tensor_copy(
                    v_sb[:, :, h * VAUG + 96:h * VAUG + 97],
                    nc.const_aps.tensor(1.0, (128, M_CH, 1), F32))

        # ---- per query group: q-proj, attention, out-proj ----
        # q-proj runs one group ahead of attention (software pipeline) so
        # its matmuls fill PE gaps while ACT paces the previous group.
        # v-proj is emitted AFTER q-proj(0): group 0's S/softmax-exp only
        # needs K and Q, so ACT starts ~20us earlier, overlapping v-proj.
        def qload(gi, gsz):
            xq = pin.tile([128, DQ_CH, 512], BF16, tag="xq", name="xq")
            nc.sync.dma_start(out=xq[:, :, :gsz], in_=xt[:, gi, :, :gsz])
            return xq

        def qproj_head(xq, h, g0, gsz):
            ps = pps.tile([128, 512], F32, tag="pp", name="pp")
            for dc in range(DQ_CH):
                nc.tensor.matmul(
                    ps[:, :gsz], wq_sb[:, dc, h * CP:(h + 1) * CP],
                    xq[:, dc, :gsz],
                    start=(dc == 0), stop=(dc == DQ_CH - 1))
            nc.vector.tensor_copy(q_heads[h][:, g0:g0 + gsz], ps[:, :gsz])

        def qproj(gi, g0, gsz):
            xq = qload(gi, gsz)
            for h in range(HPC):
                qproj_head(xq, h, g0, gsz)

        def s_exp(h, g0, gsz):
            # S chunk pairs in a 2-bank PSUM tile; one softmax-exp per pair
            # halves the ACT instruction count (ACT is the most saturated
            # engine: ~0.3us fixed bubble per activation instruction)
            expst = expp.tile([128, M_CH, 512], BF16, tag="expst",
                              name="expst")
            for t in range(M_CH // 2):
                st = stp.tile([128, 2, 512], F32, tag="st", name="st")
                for j in range(2):
                    mc = 2 * t + j
                    nc.tensor.matmul(
                        st[:, j, :gsz],
                        k_heads[h][:, mc * 128:(mc + 1) * 128],
                        q_heads[h][:, g0:g0 + gsz], start=True, stop=True)
                nc.scalar.activation(expst[:, 2 * t:2 * t + 2, :gsz],
                                     st[:, :, :gsz], EXP, scale=SCALE)
            return expst

        # ---- prologue: per head, k-proj -> q-proj(g0) -> S/exp(g0), so
        # the first softmax-exp lands on ACT ~10us in; v-proj follows. ----
        g0_0, gsz_0 = groups[0]
        xq0 = qload(0, gsz_0)
        expsts0 = []
        for h in range(HPC):
            kproj(h)
            qproj_head(xq0, h, g0_0, gsz_0)
            expsts0.append(s_exp(h, g0_0, gsz_0))

        expsts = {0: expsts0}
        pending = [None]  # deferred normalize+out-proj closure for group g-1

        def make_finisher(gi, g0, gsz, o_uns, bcast):
            """Normalize (Pool), out-project, PSUM->SBUF copy (ACT) and
            store group gi.  Runs inside group gi+1's body, after its
            h-loop: by then the reciprocal round-trip for gi has long
            landed, so nothing here blocks the DVE/ACT queues, and the
            PE matmuls act as filler between AV streams."""
            def fin():
                out_hs = []
                for h in range(HPC):
                    out_h = ap.tile([80, 512], BF16, tag=f"outH{h}",
                                    name=f"outH{h}")
                    nc.gpsimd.tensor_tensor(
                        out_h[:, :gsz], o_uns[h][:80, :gsz],
                        bcast[:, h, :gsz], MUL)
                    out_hs.append(out_h)
                ntile = gsz // 128
                ysb = ap.tile([128, 4, QD], BF16, tag="ysb", name="ysb")
                for nt0 in range(ntile):
                    for di in range(2):
                        yps = yp.tile([128, 512], F32, tag="pp", name="y")
                        for h in range(HPC):
                            nc.tensor.matmul(
                                yps[:, :320],
                                out_hs[h][:, nt0 * 128:(nt0 + 1) * 128],
                                wout_sb[:, h, di * 320:(di + 1) * 320],
                                start=(h == 0), stop=(h == HPC - 1))
                        nc.scalar.activation(
                            ysb[:, nt0, di * 320:(di + 1) * 320],
                            yps[:, :320], COPY)
                nc.sync.dma_start(out=y[:, gi, :ntile, :],
                                  in_=ysb[:, :ntile, :])
            return fin

        for gi, (g0, gsz) in enumerate(groups):
            # attention: unnormalized out + denominator per head, then one
            # batched reciprocal round-trip + partition-broadcast per group
            # (per-head PE-broadcast chains for the last group, to shorten
            # the exposed tail after the final AV)
            last = gi == len(groups) - 1
            bcast = ap.tile([80, HPC, 512], BF16, tag="bcast", name="bcast")
            rdr = dsc.tile([HPC, 512], BF16, tag="rdr", name="rdr")
            recip_all = ap.tile([1, HPC, 512], BF16, tag="recip",
                                name="recip_all")

            def av_norm(h, expst):
                oaug = ovp.tile([VAUG, 512], F32, tag="oaug", name="oaug")
                for mc in range(M_CH):
                    nc.tensor.matmul(
                        oaug[:, :gsz],
                        v_sb[:, mc, h * VAUG:(h + 1) * VAUG],
                        expst[:, mc, :gsz],
                        start=(mc == 0), stop=(mc == M_CH - 1))
                o_un = ap.tile([80, 512], F32, tag=f"oun{h}", name=f"oun{h}")
                nc.vector.tensor_copy(o_un[:, :gsz], oaug[:80, :gsz])
                with nc.allow_low_precision(reason="bf16 softmax recip"):
                    nc.vector.reciprocal(recip_all[:, h, :gsz],
                                         oaug[96:97, :gsz])
                o_uns.append(o_un)

            def s_exp_av(h, g0n, gszn, expst_cur):
                # next-group S/exp interleaved chunk-wise with the current
                # group's AV accumulation: the AV matmuls (whose exp inputs
                # were produced a whole group ago) are guaranteed PE filler
                # between S chunks, so the in-order PE queue never idles at
                # the softmax-exp pace of the stp-buffer rotation.
                expst = expp.tile([128, M_CH, 512], BF16, tag="expst",
                                  name="expst")
                oaug = ovp.tile([VAUG, 512], F32, tag="oaug", name="oaug")
                for t in range(M_CH // 2):
                    st = stp.tile([128, 2, 512], F32, tag="st", name="st")
                    for j in range(2):
                        mc = 2 * t + j
                        nc.tensor.matmul(
                            st[:, j, :gszn],
                            k_heads[h][:, mc * 128:(mc + 1) * 128],
                            q_heads[h][:, g0n:g0n + gszn],
                            start=True, stop=True, skip_group_check=True)
                        nc.tensor.matmul(
                            oaug[:, :gsz],
                            v_sb[:, mc, h * VAUG:(h + 1) * VAUG],
                            expst_cur[:, mc, :gsz],
                            start=(mc == 0), stop=(mc == M_CH - 1),
                            skip_group_check=True)
                    nc.scalar.activation(expst[:, 2 * t:2 * t + 2, :gszn],
                                         st[:, :, :gszn], EXP, scale=SCALE)
                o_un = ap.tile([80, 512], F32, tag=f"oun{h}",
                               name=f"oun{h}")
                nc.vector.tensor_copy(o_un[:, :gsz], oaug[:80, :gsz])
                with nc.allow_low_precision(reason="bf16 softmax recip"):
                    nc.vector.reciprocal(recip_all[:, h, :gsz],
                                         oaug[96:97, :gsz])
                o_uns.append(o_un)
                return expst

            def emit_bc(h):
                # PE rank-1 broadcast: PSUM[80,512] = ones^T @ recip —
                # PE is idle in the tail and this skips the DRAM
                # round-trip on the exposed critical path
                bc = ovp.tile([128, 512], F32, tag="oaug", name="bc")
                nc.tensor.matmul(
                    bc[:80, :gsz],
                    nc.const_aps.tensor(1.0, (1, 80), BF16),
                    recip_all[:, h, :gsz], start=True, stop=True)
                bcs.append(bc)

            # the deferred finisher for the previous group must run BEFORE
            # the tailpipe grabs both out-proj PSUM slots (deadlock), and by
            # the last group its reciprocal broadcast has long arrived
            if last and pending[0] is not None:
                pending[0]()
                pending[0] = None

            # last-group tail pipeline (ntile==1): after AV(h), finish head
            # h-1 (bc -> mul -> its two yout accumulation matmuls), so the
            # recip->bc->mul chain of head h overlaps AV(h+1) instead of
            # stalling PE, and only head 3's chain is tail-exposed.
            tailpipe = last and gsz == 128
            yps_d = ([yp.tile([128, 512], F32, tag="pp", name="y")
                      for _ in range(2)] if tailpipe else None)
            out_hs = []

            def finish_head(h):
                out_h = ap.tile([80, 512], BF16, tag=f"outH{h}",
                                name=f"outH{h}")
                nc.vector.tensor_tensor(
                    out_h[:, :gsz], o_uns[h][:80, :gsz],
                    bcs[h][:80, :gsz], MUL)
                out_hs.append(out_h)
                for di in range(2):
                    nc.tensor.matmul(
                        yps_d[di][:, :320], out_h[:, :128],
                        wout_sb[:, h, di * 320:(di + 1) * 320],
                        start=(h == 0), stop=(h == HPC - 1),
                        skip_group_check=True)

            o_uns, bcs = [], []
            if gi == 0:
                # S/exp for group 0 was emitted in the prologue; v-proj
                # (which AV needs) comes next, overlapping those exps,
                # then q-proj for group 1.
                vproj()
                if len(groups) > 1:
                    qproj(1, *groups[1])
            # software pipeline: S/exp for group g+1 interleaves per-head
            # with AV of group g on the in-order PE queue; the deferred
            # finisher for group g-1 and q-proj for g+2 follow as filler.
            if gi + 1 < len(groups):
                expsts[gi + 1] = []
                g1, s1 = groups[gi + 1]
            for h in range(HPC):
                if gi + 1 < len(groups):
                    expsts[gi + 1].append(
                        s_exp_av(h, g1, s1, expsts[gi][h]))
                else:
                    av_norm(h, expsts[gi][h])
                if tailpipe and h > 0:
                    emit_bc(h - 1)
                    finish_head(h - 1)
            if not last:
                nc.sync.dma_start(out=rdr[:, :gsz],
                                  in_=recip_all[:, :, :gsz])
                nc.sync.dma_start(out=bcast[:, :, :gsz],
                                  in_=_pbcast(rdr[:, :gsz], 80))
            if pending[0] is not None:
                pending[0]()
                pending[0] = None
            if gi + 2 < len(groups):
                qproj(gi + 2, *groups[gi + 2])

            if tailpipe:
                ysb = ap.tile([128, 4, QD], BF16, tag="ysb", name="ysb")
                emit_bc(HPC - 1)
                finish_head(HPC - 1)
                for di in range(2):
                    nc.scalar.activation(
                        ysb[:, 0, di * 320:(di + 1) * 320],
                        yps_d[di][:, :320], COPY)
                nc.sync.dma_start(out=y[:, gi, 0:1, :], in_=ysb[:, 0:1, :])
            elif last:
                # non-128 last group: finish inline via PE broadcasts
                ysb = ap.tile([128, 4, QD], BF16, tag="ysb", name="ysb")
                for h in range(HPC):
                    emit_bc(h)
                    out_h = ap.tile([80, 512], BF16, tag=f"outH{h}",
                                    name=f"outH{h}")
                    nc.vector.tensor_tensor(
                        out_h[:, :gsz], o_uns[h][:80, :gsz],
                        bcs[h][:80, :gsz], MUL)
                    out_hs.append(out_h)
                ntile = gsz // 128
                for nt0 in range(ntile):
                    for di in range(2):
                        yps = yp.tile([128, 512], F32, tag="pp", name="y")
                        for h in range(HPC):
                            nc.tensor.matmul(
                                yps[:, :320],
                                out_hs[h][:, nt0 * 128:(nt0 + 1) * 128],
                                wout_sb[:, h, di * 320:(di + 1) * 320],
                                start=(h == 0), stop=(h == HPC - 1))
                        nc.scalar.activation(
                            ysb[:, nt0, di * 320:(di + 1) * 320],
                            yps[:, :320], COPY)
                    nc.sync.dma_start(out=y[:, gi, nt0:nt0 + 1, :],
                                      in_=ysb[:, nt0:nt0 + 1, :])
            else:
                pending[0] = make_finisher(gi, g0, gsz, o_uns, bcast)
        assert pending[0] is None


def _pm(a, nchunk, p=128):
    """[nchunk*p, F...] -> partition-major [p, nchunk, F...]."""
    return np.ascontiguousarray(
        a.reshape(nchunk, p, *a.shape[1:]).transpose(
            1, 0, *range(2, a.ndim + 1)))


def _prep_core_inputs(x, key, value, wq, wk, wv, wout, bout,
                      qmask_idx, npad):
    """Host-side shard prep: returns list of 8 in_maps (bf16, partition-
    major layouts matching the DRAM tensor declarations in build())."""
    ngr = -(-npad // 512)
    in_maps = []
    xt_b, kt_b, vt_b = {}, {}, {}
    for b in range(B):
        idx = qmask_idx[b]
        xs = np.zeros((QD, ngr * 512), dtype=bfloat16)
        xs[:, :len(idx)] = np.ascontiguousarray(x[b][idx].T).astype(bfloat16)
        # [640, ngr*512] -> [5, 128, ngr, 512] -> [128, ngr, 5, 512]
        xt_b[b] = np.ascontiguousarray(
            xs.reshape(DQ_CH, 128, ngr, 512).transpose(1, 2, 0, 3))
        kb = np.ascontiguousarray(key[b].T).astype(bfloat16)
        kt_b[b] = np.ascontiguousarray(
            kb.reshape(DK_CH, 128, 2, 512).transpose(1, 2, 0, 3))
        vb = np.ascontiguousarray(value[b].T).astype(bfloat16)
        vt_b[b] = np.ascontiguousarray(
            vb.reshape(DK_CH, 128, M_CH, 128).transpose(1, 2, 0, 3))

    w_half = {}
    for hh in range(2):
        wq_a = np.zeros((QD, HPC * CP), dtype=bfloat16)
        wk_a = np.zeros((KD, HPC * CP), dtype=bfloat16)
        wv_a = np.zeros((KD, HPC * 80), dtype=bfloat16)
        for hp in range(HPC):
            hg = hh * HPC + hp
            wq_a[:, hp * CP:hp * CP + 80] = wq[:, hg * 80:(hg + 1) * 80].astype(bfloat16)
            wk_a[:, hp * CP:hp * CP + 80] = wk[:, hg * 80:(hg + 1) * 80].astype(bfloat16)
            wv_a[:, hp * 80:(hp + 1) * 80] = wv[:, hg * 80:(hg + 1) * 80].astype(bfloat16)
        w_half[hh] = (_pm(wq_a, DQ_CH), _pm(wk_a, DK_CH), _pm(wv_a, DK_CH))

    for core in range(8):
        b, hh = core // 2, core % 2
        wq_a, wk_a, wv_a = w_half[hh]
        wo = wout[hh * HPC * 80:(hh + 1) * HPC * 80].astype(bfloat16)
        wout_a = np.ascontiguousarray(
            wo.reshape(HPC, 80, QD).transpose(1, 0, 2))
        in_maps.append({
            "xt": xt_b[b], "kt": kt_b[b], "vt": vt_b[b],
            "wq": wq_a, "wk": wk_a, "wv": wv_a, "wout": wout_a,
        })
    return in_maps


def kernel(x, key, value, box_mask, Wq, Wk, Wv, Wout, bout, _trace=False):
    x = np.asarray(x, dtype=np.float32)
    key = np.asarray(key, dtype=np.float32)
    value = np.asarray(value, dtype=np.float32)
    box_mask = np.asarray(box_mask, dtype=np.float32)
    Wq, Wk, Wv = (np.asarray(a, dtype=np.float32) for a in (Wq, Wk, Wv))
    Wout = np.asarray(Wout, dtype=np.float32)
    bout = np.asarray(bout, dtype=np.float32)

    qmask = box_mask[:, 0].reshape(B, N) > 0.5
    qmask_idx = [np.nonzero(qmask[b])[0] for b in range(B)]
    cnt_max = max(1, max(len(i) for i in qmask_idx))
    npad = -(-cnt_max // 128) * 128

    nc = build(npad)
    in_maps = _prep_core_inputs(x, key, value, Wq, Wk, Wv, Wout, bout,
                                qmask_idx, npad)
    # Device execution with transient-corruption guard: dispatch twice and
    # accept only when two runs agree bitwise (HW is deterministic on clean
    # runs; a glitched run differs).  Retry a few times on mismatch.
    kr = run_bass_kernel_spmd(nc, in_maps, core_ids=list(range(8)),
                              trace=_trace)
    results = kr.results
    if not _trace:
        prev = [r["y"].copy() for r in results]
        for _attempt in range(4):
            kr2 = run_bass_kernel_spmd(nc, in_maps, core_ids=list(range(8)))
            cur = [r["y"] for r in kr2.results]
            if all(np.array_equal(a, b) for a, b in zip(prev, cur)):
                results = kr2.results
                break
            prev = [c.copy() for c in cur]
        else:
            results = kr2.results

    out = np.broadcast_to(bout, (B, N, QD)).copy().astype(np.float32)
    for b in range(B):
        idx = qmask_idx[b]
        # y is [128, ngr, 4, QD]: query g*512 + nt0*128 + p lives at
        # y[p, g, nt0]
        y0 = results[2 * b]["y"].transpose(1, 2, 0, 3).reshape(-1, QD)
        y1 = results[2 * b + 1]["y"].transpose(1, 2, 0, 3).reshape(-1, QD)
        out[b][idx] = (y0[:len(idx)].astype(np.float32)
                       + y1[:len(idx)].astype(np.float32) + bout)
    if _trace:
        return out, kr
    return out

